# revision 1
# baseline (speedup 1.0000x reference)
"""CCA (cross-covariance / channel) attention kernel for Trainium2, 8 NeuronCores.

Math (per batch element b, all derived from the reference nn.Module):
    qkv = x @ W_qkv ; per head h: q,k,v in [N, 64] layouts
    channel attention: attn_h = softmax_d( (q_hat^T k_hat) * temp_h ),
    with q_hat = q / ||q||_col (L2 over N), out = attn @ v^T, y = out^T @ W_proj + b.

Key factorization used here (N=4096 >> C=512):
    S = x^T x                      [512,512]   (shared across heads)
    qk_h = Wq_h^T S Wk_h,  |q_c|^2 = diag(Wq_h^T S Wq_h)  (via T = S @ Wqk)
    M_h = attn_h^T Wp_h            [64,512]
    P   = sum_h Wv_h M_h           [512,512]
    y   = x @ P + b                 (big matmul, uses host-pretransposed x^T)

Data-parallel over B=8 across the 8 cores; no collectives.
"""

import os
import sys
import numpy as np

for _p in ("/opt/trn_rl_repo",):
    if _p not in sys.path and os.path.isdir(_p):
        sys.path.insert(0, _p)

import ml_dtypes  # noqa: E402
from contextlib import ExitStack  # noqa: E402

import functools  # noqa: E402

import concourse.bass as bass  # noqa: E402
import concourse.bacc as bacc  # noqa: E402
import concourse.hw_specs as hw_specs  # noqa: E402


@functools.cache
def _patched_act_tables(arch):
    # Keep Ln/Exp only in natural_log_exp_and_others so the table-load pass
    # resolves both to ONE set (a single ~1.3us ACT_TABLE_LOAD per kernel).
    base = hw_specs.get_activation_tables(arch)
    out = {}
    for name, fns in base.items():
        fns = set(fns)
        if name != "natural_log_exp_and_others":
            fns -= {mybir.ActivationFunctionType.Ln, mybir.ActivationFunctionType.Exp}
        out[name] = fns
    return out


bacc.get_activation_tables = _patched_act_tables
import concourse.tile as tile  # noqa: E402
from concourse import mybir  # noqa: E402
from concourse.bass_utils import run_bass_kernel_spmd  # noqa: E402
from concourse.tile_rust import add_dep_helper  # noqa: E402

B, N, C = 8, 4096, 512
NH, HD = 8, 64
NT = N // 128  # 32 n-tiles
KC = C // 128  # 4 contraction chunks of 128
F32 = mybir.dt.float32
BF16 = mybir.dt.bfloat16
FP8 = mybir.dt.float8e4
AF = mybir.ActivationFunctionType
ALU = mybir.AluOpType
BF16_NP = ml_dtypes.bfloat16
FP8_NP = ml_dtypes.float8_e4m3


def _build_kernel_body(ctx: ExitStack, tc: tile.TileContext, io: dict):
    nc = tc.nc
    x_nat, x_tr, wqk, wvt, wp, bpr, temp, y = (
        io["x_nat"], io["x_tr"], io["wqk"], io["wvt"], io["wp"],
        io["bpr"], io["temp"], io["y"],
    )

    persist = ctx.enter_context(tc.tile_pool(name="persist", bufs=1))
    ypool = ctx.enter_context(tc.tile_pool(name="ypool", bufs=6))
    psum = ctx.enter_context(tc.tile_pool(name="psum", bufs=6, space="PSUM"))
    psum_g = ctx.enter_context(tc.tile_pool(name="psum_g", bufs=1, space="PSUM"))

    # ---- loads -------------------------------------------------------------
    # x (fp8, feeds only S) is host-pre-tiled to [128, NT, C] so every DMA
    # reads long contiguous per-partition lines; streamed in 4 chunks
    # alternating between the two HWDGE queues.
    CHUNK_TILES = [16, 16]
    NCHUNK = len(CHUNK_TILES)
    x_chunks = []
    t0 = 0
    for c, ntc in enumerate(CHUNK_TILES):
        xc = persist.tile([128, ntc, C], FP8, tag=f"x_chunk{c}")
        nc.sync.dma_start(out=xc, in_=x_nat[:, t0:t0 + ntc, :])
        x_chunks.append(xc)
        t0 += ntc
    wqk_sb = persist.tile([128, KC, 2 * C], BF16)
    nc.scalar.dma_start(out=wqk_sb, in_=wqk[:].rearrange("(k p) c -> p k c", p=128))
    wvt_sb = persist.tile([64, NH, C], BF16)  # [d, (h, ci)]
    nc.gpsimd.dma_start(out=wvt_sb, in_=wvt[:].rearrange("(h d) c -> d h c", h=NH))
    wp_sb = persist.tile([64, NH, C], BF16)  # [c, (h, e)]
    nc.gpsimd.dma_start(out=wp_sb, in_=wp[:].rearrange("(h c) e -> c h e", h=NH))
    bias_sb = persist.tile([128, C], F32)
    nc.gpsimd.dma_start(
        out=bias_sb,
        in_=bass.AP(tensor=bpr[:].tensor, offset=bpr[:].offset, ap=[[0, 128], [1, C]]),
    )
    temp_b = persist.tile([128, NH], F32)
    nc.gpsimd.dma_start(
        out=temp_b,
        in_=bass.AP(tensor=temp[:].tensor, offset=temp[:].offset,
                    ap=[[0, 128], [1, NH]]),
    )
    ones_sb = persist.tile([128, 1], BF16)
    nc.vector.memset(ones_sb, 1.0)
    # xT is only needed by the final y phase; chunk by n-range so the first
    # y tiles can start before the whole 4MB lands.
    xt_sb = persist.tile([128, KC, N], BF16)
    xt_view = x_tr[:].rearrange("(k p) n -> p k n", p=128)
    xt_dmas = []
    for g in range(8):
        xt_dmas.append(nc.gpsimd.dma_start(
            out=xt_sb[:, :, g * 512:(g + 1) * 512],
            in_=xt_view[:, :, g * 512:(g + 1) * 512],
        ))

    # ACT table warmup during the S phase. Order matters: Exp first, Ln last,
    # so the Ln set is resident when the norms chain starts.
    warm_sb = persist.tile([1, 2], F32)
    nc.vector.memset(warm_sb, 1.0)
    nc.scalar.activation(warm_sb[:, 1:2], warm_sb[:, 1:2], AF.Exp)
    nc.scalar.activation(warm_sb[:, 0:1], warm_sb[:, 0:1], AF.Ln)

    # PE keepalive helper: a tiny matmul dependent on `dep` keeps the HAM
    # activity monitor from re-throttling the PE during compute-idle windows.
    _keep_n = [0]

    def keep(dep):
        kp = psum.tile([1, 2], F32, tag="work_ps", name=f"keep{_keep_n[0]}")
        _keep_n[0] += 1
        nc.tensor.matmul(kp[:, 0:1], dep, dep, start=True, stop=True)

    def dense(n):
        # dependency-paced full-width dummy matmuls: real PE density to keep
        # the HAM clock gate at 8/8 through compute-idle windows.
        for _ in range(n):
            kp = psum.tile([128, C], F32, tag="work_ps", name=f"dense{_keep_n[0]}")
            _keep_n[0] += 1
            nc.tensor.matmul(
                kp, wqk_sb[:, 0, 0:128], wqk_sb[:, 0, 0:C], start=True, stop=True
            )

    # PE pre-warm: dependency-free full-width dummy matmuls run during the
    # initial DMA wait, so the HAM clock gate is at 8/8 when S starts.
    scr_sb = persist.tile([128, C], BF16)
    nc.vector.memset(scr_sb, 1.0)
    for i in range(12):
        kp = psum.tile([128, C], F32, tag="work_ps", name=f"prewarm{i}")
        nc.tensor.matmul(kp, scr_sb[:, 0:128], scr_sb, start=True, stop=True)

    # ---- S = x^T x  [C, C] -------------------------------------------------
    # chunk-outer loop so accumulation starts when the first x chunk arrives;
    # the 4 S psum tiles stay live across chunks.
    s_sb = persist.tile([128, KC, C], BF16)
    s_ps = [
        psum.tile([128, C], F32, tag="work_ps", name=f"s_ps{kc}") for kc in range(KC)
    ]
    # fp8 DoubleRow: each matmul consumes a pair of 128-row n-tiles
    # (lhsT [128, 2, 128], rhs [128, 2, 512] -> out [128, 512]).
    last_s_mm = None
    for kc in range(KC):
        for c, ntc in enumerate(CHUNK_TILES):
            for tp in range(ntc // 2):
                last_s_mm = nc.tensor.matmul(
                    s_ps[kc],
                    x_chunks[c][:, 2 * tp:2 * tp + 2, kc * 128:(kc + 1) * 128],
                    x_chunks[c][:, 2 * tp:2 * tp + 2, :],
                    perf_mode=mybir.MatmulPerfMode.DoubleRow,
                    start=(c == 0 and tp == 0),
                    stop=(c == NCHUNK - 1 and tp == ntc // 2 - 1),
                )
        nc.vector.tensor_copy(s_sb[:, kc, :], s_ps[kc])
    # defer the xT transfers until x has fully landed: keeps the front HBM
    # bandwidth dedicated to the S-phase inputs
    for xd in xt_dmas:
        add_dep_helper(xd.ins, last_s_mm.ins,
                       reason="xT load deferred behind S inputs")

    # ---- T = S @ Wqk [C, 2C], with norms^2 accumulation interleaved -------
    # pn = Wqk*T (DVE) and the ones-matmul rows run per T-row so the whole
    # norms chain can start the moment T finishes.
    t_sb = persist.tile([128, KC, 2 * C], BF16)
    pn_sb = persist.tile([128, KC, 2 * C], BF16)
    nrm_ps = [
        psum.tile([1, C], F32, tag="work_ps", name=f"nrm_ps{half}")
        for half in range(2)
    ]
    for ti in range(KC):
        for half in range(2):
            t_ps = psum.tile([128, C], F32, tag="work_ps")
            for kj in range(KC):
                nc.tensor.matmul(
                    t_ps,
                    s_sb[:, kj, ti * 128:(ti + 1) * 128],
                    wqk_sb[:, kj, half * C:(half + 1) * C],
                    start=(kj == 0),
                    stop=(kj == KC - 1),
                )
            nc.scalar.copy(t_sb[:, ti, half * C:half * C + 256], t_ps[:, 0:256])
            nc.vector.tensor_copy(
                t_sb[:, ti, half * C + 256:(half + 1) * C], t_ps[:, 256:C]
            )
        nc.vector.tensor_mul(pn_sb[:, ti, :], wqk_sb[:, ti, :], t_sb[:, ti, :])
        for half in range(2):
            nc.tensor.matmul(
                nrm_ps[half],
                ones_sb,
                pn_sb[:, ti, half * C:(half + 1) * C],
                start=(ti == 0),
                stop=(ti == KC - 1),
            )

    # ---- norms chain: r = 1/||.|| = exp(-0.5*ln(n^2)) ---------------------
    # ACT-only (DVE reciprocal on a single-partition row costs ~6.5us).
    # Norms are ~64 here, so the reference's max(.,1e-12) clamp is inert.
    # Emitted BEFORE G so the chain overlaps the G matmuls below.
    lnr = persist.tile([1, 2 * C], F32)
    r_row = persist.tile([1, 2 * C], BF16)
    keep_chain = []
    for half in range(2):
        nc.scalar.activation(lnr[:, half * C:(half + 1) * C], nrm_ps[half], AF.Ln)
        keep_chain.append(lnr[0:1, half * C:half * C + 1])
    for half in range(2):
        nc.scalar.activation(
            r_row[:, half * C:(half + 1) * C],
            lnr[:, half * C:(half + 1) * C],
            AF.Exp,
            scale=-0.5,
        )
        keep_chain.append(r_row[0:1, half * C + 1:half * C + 2])

    # ---- G_h = Wqk_h^T T_h  [128, 128] per head ---------------------------
    # (independent of the norms chain; keeps the PE warm while ACT works)
    g_ps = psum_g.tile([128, NH, 128], F32)
    for h in range(NH):
        for kc in range(KC):
            nc.tensor.matmul(
                g_ps[:, h, :],
                wqk_sb[:, kc, h * 128:(h + 1) * 128],
                t_sb[:, kc, h * 128:(h + 1) * 128],
                start=(kc == 0),
                stop=(kc == KC - 1),
            )

    for dep in keep_chain:
        keep(dep)
        dense(3)

    # r -> per-partition rq via 8 tiny PE transposes (no DRAM bounce), and
    # free-dim-broadcast rk via a K=1 outer-product matmul.
    ident1 = persist.tile([1, 1], BF16)
    nc.vector.memset(ident1, 1.0)
    ones64 = persist.tile([1, HD], BF16)
    nc.vector.memset(ones64, 1.0)
    tr_ps = psum.tile([128, 2 * NH], BF16, tag="work_ps")
    for h in range(NH):
        nc.tensor.transpose(
            tr_ps[:, 2 * h:2 * h + 1], r_row[0:1, h * 128:(h + 1) * 128], ident1
        )
    rq_sb = persist.tile([128, NH], F32)
    tr_view = tr_ps.rearrange("p (h two) -> p h two", two=2)[:, :, 0]
    nc.vector.tensor_mul(rq_sb, tr_view, temp_b)  # fold temperature into rq
    rk_ps = psum.tile([64, C], F32, tag="work_ps")
    rk_src = r_row.rearrange("p (h s) -> p h s", s=128)[:, :, HD:]
    nc.tensor.matmul(rk_ps, ones64, rk_src, start=True, stop=True)
    rk_sb = persist.tile([64, NH, HD], F32)
    nc.scalar.copy(rk_sb, rk_ps)
    keep(rk_sb[0:1, 0, 0:1])
    dense(3)

    # ---- softmax -> M_h -> P, in two head-groups ---------------------------
    # |logits| <= max(temperature) so exp() is safe without max-subtraction.
    # Stage loops within each group keep every engine streaming; group 1's
    # softmax overlaps group 0's M/P matmuls, which keeps the PE warm.
    GH = NH // 2
    lg = persist.tile([64, NH, HD], F32)
    ex = persist.tile([64, NH, HD], F32)
    ssum = persist.tile([64, NH], F32)
    attn = persist.tile([64, NH, HD], BF16)
    m_sb = persist.tile([64, NH, C], BF16)
    p_ps = [
        psum.tile([128, C], F32, tag="work_ps", name=f"p_ps{t}") for t in range(KC)
    ]
    def emit_p(h):
        for t in range(KC):
            nc.tensor.matmul(
                p_ps[t],
                wvt_sb[:, h, t * 128:(t + 1) * 128],
                m_sb[:, h, :],
                start=(h == 0),
                stop=(h == NH - 1),
            )

    for grp in range(2):
        h0 = grp * GH
        for h in range(h0, h0 + GH):
            nc.vector.scalar_tensor_tensor(
                out=lg[:, h, :],
                in0=g_ps[0:64, h, HD:128],
                scalar=rq_sb[0:64, h:h + 1],
                in1=rk_sb[:, h, :],
                op0=ALU.mult,
                op1=ALU.mult,
            )
        keep(lg[0:1, h0 + 1, 0:1])
        dense(2)
        keep(lg[0:1, h0 + GH - 1, 0:1])
        dense(2)
        nc.scalar.activation(
            ex[:, h0:h0 + GH, :], lg[:, h0:h0 + GH, :], AF.Exp
        )
        keep(ex[0:1, h0 + 1, 0:1])
        dense(2)
        nc.vector.tensor_reduce(
            ssum[:, h0:h0 + GH, None], ex[:, h0:h0 + GH, :],
            axis=mybir.AxisListType.X, op=ALU.add,
        )
        nc.vector.reciprocal(ssum[:, h0:h0 + GH], ssum[:, h0:h0 + GH])
        nc.vector.tensor_mul(
            attn[:, h0:h0 + GH, :],
            ex[:, h0:h0 + GH, :],
            ssum[:, h0:h0 + GH, None].broadcast_to([64, GH, HD]),
        )
        # M/P software pipeline: P(h-1) runs while m_sb[h] is being copied
        for h in range(h0, h0 + GH):
            m_ps = psum.tile([64, C], F32, tag="work_ps")
            nc.tensor.matmul(
                m_ps, attn[:, h, :], wp_sb[:, h, :], start=True, stop=True
            )
            if h % 2 == 0:
                nc.scalar.copy(m_sb[:, h, :], m_ps)
            else:
                nc.vector.tensor_copy(m_sb[:, h, :], m_ps)
            if h > h0:
                emit_p(h - 1)
        emit_p(h0 + GH - 1)

    p_sb = persist.tile([128, KC, C], BF16)
    for t in range(KC):
        nc.scalar.copy(p_sb[:, t, :], p_ps[t])

    # ---- y = x @ P + b ----------------------------------------------------
    y_tiled = y[:].rearrange("(t p) c -> t p c", p=128)
    for t in range(NT):
        y_ps = psum.tile([128, C], F32, tag="work_ps")
        for kc in range(KC):
            nc.tensor.matmul(
                y_ps,
                xt_sb[:, kc, t * 128:(t + 1) * 128],
                p_sb[:, kc, :],
                start=(kc == 0),
                stop=(kc == KC - 1),
            )
        y_t = ypool.tile([128, C], BF16)
        nc.vector.tensor_add(y_t, y_ps, bias_sb)
        eng = nc.sync if t % 2 == 0 else nc.scalar
        eng.dma_start(out=y_tiled[t], in_=y_t)


def build_nc():
    nc = bacc.Bacc("TRN2", target_bir_lowering=False, debug=False, num_devices=B)
    io = {}
    io["x_nat"] = nc.dram_tensor("x_nat", [128, NT, C], FP8, kind="ExternalInput")
    io["x_tr"] = nc.dram_tensor("x_tr", [C, N], BF16, kind="ExternalInput")
    io["wqk"] = nc.dram_tensor("wqk", [C, 2 * C], BF16, kind="ExternalInput")
    io["wvt"] = nc.dram_tensor("wvt", [C, C], BF16, kind="ExternalInput")
    io["wp"] = nc.dram_tensor("wp", [C, C], BF16, kind="ExternalInput")
    io["bpr"] = nc.dram_tensor("bpr", [C], F32, kind="ExternalInput")
    io["temp"] = nc.dram_tensor("temp", [NH], F32, kind="ExternalInput")
    io["y"] = nc.dram_tensor("y", [N, C], BF16, kind="ExternalOutput")
    with tile.TileContext(nc) as tc:
        with ExitStack() as ctx:
            _build_kernel_body(ctx, tc, io)
    nc.compile()
    return nc


_NC_CACHE = None


def _get_nc():
    global _NC_CACHE
    if _NC_CACHE is None:
        _NC_CACHE = build_nc()
    return _NC_CACHE


def prep_host_inputs(x, W_qkv, temperature, W_proj, b_proj):
    """Host-side preprocessing shared by all cores. Returns per-core in_maps."""
    x = np.asarray(x, dtype=np.float32)
    W_qkv = np.asarray(W_qkv, dtype=np.float32)
    temperature = np.asarray(temperature, dtype=np.float32).reshape(NH)
    W_proj = np.asarray(W_proj, dtype=np.float32)
    b_proj = np.asarray(b_proj, dtype=np.float32)

    Wq = W_qkv[:, 0:C].reshape(C, NH, HD)
    Wk = W_qkv[:, C:2 * C].reshape(C, NH, HD)
    wqk_perm = np.concatenate([Wq, Wk], axis=2).reshape(C, 2 * C)  # [(ci),(h)(qk c)]
    Wv = W_qkv[:, 2 * C:3 * C]  # [ci, (h d)]

    wqk_bf = np.ascontiguousarray(wqk_perm).astype(BF16_NP)
    wvt_bf = np.ascontiguousarray(Wv.T).astype(BF16_NP)
    wp_bf = np.ascontiguousarray(W_proj).astype(BF16_NP)

    in_maps = []
    for b in range(B):
        xb = x[b]
        in_maps.append({
            "x_nat": np.ascontiguousarray(
                xb.reshape(NT, 128, C).transpose(1, 0, 2)
            ).astype(FP8_NP),
            "x_tr": np.ascontiguousarray(xb.T).astype(BF16_NP),
            "wqk": wqk_bf,
            "wvt": wvt_bf,
            "wp": wp_bf,
            "bpr": b_proj,
            "temp": temperature,
        })
    return in_maps


def kernel(**inputs):
    x = inputs["x"]
    in_maps = prep_host_inputs(
        x, inputs["W_qkv"], inputs["temperature"], inputs["W_proj"], inputs["b_proj"]
    )
    nc = _get_nc()
    res = run_bass_kernel_spmd(nc, in_maps, list(range(B)))
    y = np.stack([np.asarray(res.results[i]["y"]) for i in range(B)], axis=0)
    return y.astype(np.float32)


if __name__ == "__main__":
    # smoke test with random data
    rng = np.random.default_rng(0)
    ins = {
        "x": rng.standard_normal((B, N, C), dtype=np.float32),
        "x_out": rng.standard_normal((B, N, C), dtype=np.float32),
        "W_qkv": (rng.standard_normal((C, 3 * C), dtype=np.float32) / np.sqrt(C)),
        "temperature": np.ones((NH, 1, 1), np.float32),
        "W_proj": (rng.standard_normal((C, C), dtype=np.float32) / np.sqrt(C)),
        "b_proj": rng.standard_normal((C,), dtype=np.float32) * 0.01,
        "H": 64,
        "W": 64,
    }
    out = kernel(**ins)
    print("out", out.shape, out.dtype, float(np.abs(out).max()))



# revision 6
# speedup vs baseline: 1.3013x; 1.3013x over previous
"""CCA (cross-covariance / channel) attention kernel for Trainium2, 8 NeuronCores.

Math (per batch element b, all derived from the reference nn.Module):
    qkv = x @ W_qkv ; per head h: q,k,v in [N, 64] layouts
    channel attention: attn_h = softmax_d( (q_hat^T k_hat) * temp_h ),
    with q_hat = q / ||q||_col (L2 over N), out = attn @ v^T, y = out^T @ W_proj + b.

Key factorization used here (N=4096 >> C=512):
    S = x^T x                      [512,512]   (shared across heads)
    qk_h = Wq_h^T S Wk_h,  |q_c|^2 = diag(Wq_h^T S Wq_h)  (via T = S @ Wqk)
    M_h = attn_h^T Wp_h            [64,512]
    P   = sum_h Wv_h M_h           [512,512]
    y   = x @ P + b                 (big matmul, uses host-pretransposed x^T)

The whole S->T->qk->softmax path is scale-invariant (the cosine
normalization cancels any uniform scale on S), so it runs in fp8
DoubleRow end to end: x is pre-scaled by 1/8 on the host so S/64 falls
out of PSUM in fp8 range with no on-chip rescale.  The y = x @ P matmul
feeds the output directly, so it stays bf16.

Data-parallel over B=8 across the 8 cores; no collectives.
"""

import os
import sys
import numpy as np

for _p in ("/opt/trn_rl_repo",):
    if _p not in sys.path and os.path.isdir(_p):
        sys.path.insert(0, _p)

import ml_dtypes  # noqa: E402
from contextlib import ExitStack  # noqa: E402

import functools  # noqa: E402

import concourse.bass as bass  # noqa: E402
import concourse.bacc as bacc  # noqa: E402
import concourse.hw_specs as hw_specs  # noqa: E402


@functools.cache
def _patched_act_tables(arch):
    # Keep Ln/Exp only in natural_log_exp_and_others so the table-load pass
    # resolves both to ONE set (a single ~1.3us ACT_TABLE_LOAD per kernel).
    base = hw_specs.get_activation_tables(arch)
    out = {}
    for name, fns in base.items():
        fns = set(fns)
        if name != "natural_log_exp_and_others":
            fns -= {mybir.ActivationFunctionType.Ln, mybir.ActivationFunctionType.Exp}
        out[name] = fns
    return out


bacc.get_activation_tables = _patched_act_tables
import concourse.tile as tile  # noqa: E402
from concourse import mybir  # noqa: E402
from concourse.bass_utils import run_bass_kernel_spmd  # noqa: E402
from concourse.tile_rust import add_dep_helper  # noqa: E402

B, N, C = 8, 4096, 512
NH, HD = 8, 64
NT = N // 128  # 32 n-tiles
KC = C // 128  # 4 contraction chunks of 128
GP = NH // 2   # 4 head-pairs for the P phase
F32 = mybir.dt.float32
BF16 = mybir.dt.bfloat16
FP8 = mybir.dt.float8e4
AF = mybir.ActivationFunctionType
ALU = mybir.AluOpType
DR = mybir.MatmulPerfMode.DoubleRow
BF16_NP = ml_dtypes.bfloat16
FP8_NP = ml_dtypes.float8_e4m3


def _build_kernel_body(ctx: ExitStack, tc: tile.TileContext, io: dict):
    nc = tc.nc
    x_nat, x_tr, wqk8, wqkb, wv2, wp2, bpr, temp, y = (
        io["x_nat"], io["x_tr"], io["wqk8"], io["wqkb"], io["wv2"],
        io["wp2"], io["bpr"], io["temp"], io["y"],
    )

    persist = ctx.enter_context(tc.tile_pool(name="persist", bufs=1))
    ypool = ctx.enter_context(tc.tile_pool(name="ypool", bufs=6))
    psum = ctx.enter_context(tc.tile_pool(name="psum", bufs=6, space="PSUM"))
    psum_g = ctx.enter_context(tc.tile_pool(name="psum_g", bufs=1, space="PSUM"))

    # ---- PE prewarm (emitted first so the tensor queue ramps the clock
    # while the first x chunk is still in flight) ---------------------------
    scr_sb = persist.tile([128, C], BF16)
    nc.vector.memset(scr_sb, 1.0)
    for i in range(6):
        kp = psum.tile([128, C], F32, tag="work_ps", name=f"prewarm{i}")
        nc.tensor.matmul(kp, scr_sb[:, 0:128], scr_sb, start=True, stop=True)

    # ---- loads -------------------------------------------------------------
    # x (fp8, pre-scaled by 1/8, feeds only S) is host-pre-tiled to
    # [128, NT, C]; streamed in 4 chunks of 8 n-tiles on the sync queue.
    NCHUNK = 4
    CT = NT // NCHUNK  # 8 tiles per chunk
    x_chunks = []
    x_dmas = []
    for c in range(NCHUNK):
        xc = persist.tile([128, CT, C], FP8, tag=f"x_chunk{c}")
        x_dmas.append(nc.sync.dma_start(out=xc, in_=x_nat[:, c * CT:(c + 1) * CT, :]))
        x_chunks.append(xc)
    wqk8_sb = persist.tile([128, KC, 2 * C], FP8)
    nc.scalar.dma_start(out=wqk8_sb, in_=wqk8[:])
    wqkb_sb = persist.tile([128, KC, 2 * C], BF16)
    nc.gpsimd.dma_start(out=wqkb_sb, in_=wqkb[:])
    wv2_sb = persist.tile([128, GP, C], BF16)  # [(two,d), pair, ci]
    nc.gpsimd.dma_start(out=wv2_sb, in_=wv2[:])
    wp_sb = persist.tile([64, NH, C], BF16)  # [c, (h, e)]
    nc.gpsimd.dma_start(out=wp_sb, in_=wp2[:])
    bias_sb = persist.tile([128, C], F32)
    nc.gpsimd.dma_start(
        out=bias_sb,
        in_=bass.AP(tensor=bpr[:].tensor, offset=bpr[:].offset, ap=[[0, 128], [1, C]]),
    )
    temp_b = persist.tile([128, NH], F32)
    nc.gpsimd.dma_start(
        out=temp_b,
        in_=bass.AP(tensor=temp[:].tensor, offset=temp[:].offset,
                    ap=[[0, 128], [1, NH]]),
    )
    ones_sb = persist.tile([128, 1], BF16)
    nc.vector.memset(ones_sb, 1.0)
    # xT (bf16, feeds only the y phase) streams behind the x chunks: each
    # sub-DMA depends on the last x chunk's DMA so the front HBM bandwidth
    # stays dedicated to the S-phase inputs.  Lands by ~18us, y needs it ~34.
    xt_sb = persist.tile([128, KC, N], BF16)
    xt_view = x_tr[:].rearrange("(k p) n -> p k n", p=128)
    for g in range(8):
        xd = nc.gpsimd.dma_start(
            out=xt_sb[:, :, g * 512:(g + 1) * 512],
            in_=xt_view[:, :, g * 512:(g + 1) * 512],
        )
        add_dep_helper(xd.ins, x_dmas[-1].ins,
                       reason="xT load deferred behind S inputs")

    # ACT table warmup. Order matters: Exp first, Ln last, so the Ln set is
    # resident when the norms chain starts.
    warm_sb = persist.tile([1, 2], F32)
    nc.vector.memset(warm_sb, 1.0)
    nc.scalar.activation(warm_sb[:, 1:2], warm_sb[:, 1:2], AF.Exp)
    nc.scalar.activation(warm_sb[:, 0:1], warm_sb[:, 0:1], AF.Ln)

    # small dependency-paced PE keepalive for the softmax/norms lulls: keep()
    # waits on the chain tensor, dense() adds real PE density behind it.
    _keep_n = [0]

    def keep(dep):
        kp = psum.tile([1, 2], F32, tag="work_ps", name=f"keep{_keep_n[0]}")
        _keep_n[0] += 1
        nc.tensor.matmul(kp[:, 0:1], dep, dep, start=True, stop=True)

    def dense(n):
        for _ in range(n):
            kp = psum.tile([128, C], F32, tag="work_ps", name=f"dense{_keep_n[0]}")
            _keep_n[0] += 1
            nc.tensor.matmul(
                kp, wqk8_sb[:, 0, 0:128], wqk8_sb[:, 0, 0:C], start=True, stop=True
            )

    # ---- S = (x/8)^T (x/8) = S_true/64  [C, C], fp8 DoubleRow -------------
    # chunk-outer loop so accumulation starts when the first x chunk arrives.
    s8_sb = persist.tile([128, KC, C], FP8)
    s_ps = [
        psum.tile([128, C], F32, tag="work_ps", name=f"s_ps{kc}") for kc in range(KC)
    ]
    for c in range(NCHUNK):
        for kc in range(KC):
            for tp in range(CT // 2):
                nc.tensor.matmul(
                    s_ps[kc],
                    x_chunks[c][:, 2 * tp:2 * tp + 2, kc * 128:(kc + 1) * 128],
                    x_chunks[c][:, 2 * tp:2 * tp + 2, :],
                    perf_mode=DR,
                    start=(c == 0 and tp == 0),
                    stop=(c == NCHUNK - 1 and tp == CT // 2 - 1),
                )
    # S -> fp8 (no rescale needed; host pre-scaled x).  Split across ACT and
    # DVE so the S->T critical path is ~2 copies, not 4.
    nc.scalar.copy(s8_sb[:, 0, :], s_ps[0])
    nc.vector.tensor_copy(s8_sb[:, 1, :], s_ps[1])
    nc.scalar.copy(s8_sb[:, 2, :], s_ps[2])
    nc.vector.tensor_copy(s8_sb[:, 3, :], s_ps[3])

    # ---- T = S8 @ Wqk8 [C, 2C] in fp8 DoubleRow; pn = Wqk*T on DVE --------
    # t8 feeds the qk matmuls (fp8); pn (read straight from PSUM) feeds the
    # norms reduction.
    t8_sb = persist.tile([128, KC, 2 * C], FP8)
    pn_sb = persist.tile([128, KC, 2 * C], BF16)
    for ti in range(KC):
        for half in range(2):
            t_ps = psum.tile([128, C], F32, tag="work_ps")
            for jp in range(2):
                nc.tensor.matmul(
                    t_ps,
                    s8_sb[:, 2 * jp:2 * jp + 2, ti * 128:(ti + 1) * 128],
                    wqk8_sb[:, 2 * jp:2 * jp + 2, half * C:(half + 1) * C],
                    perf_mode=DR,
                    start=(jp == 0),
                    stop=(jp == 1),
                )
            if half == 0:
                nc.scalar.copy(t8_sb[:, ti, 0:C], t_ps)
            else:
                nc.vector.tensor_copy(t8_sb[:, ti, C:2 * C], t_ps)
            nc.vector.tensor_mul(
                pn_sb[:, ti, half * C:(half + 1) * C],
                wqkb_sb[:, ti, half * C:(half + 1) * C],
                t_ps,
            )

    # ---- qk_h = Wq8_h^T T8_k(h)  [64, 64] per head, fp8 DoubleRow ---------
    # (only the q^T-by-k block is needed; norms come from pn below)
    qk_ps = psum_g.tile([64, NH, HD], F32)
    for jp in range(2):
        for h in range(NH):
            nc.tensor.matmul(
                qk_ps[:, h, :],
                wqk8_sb[:, 2 * jp:2 * jp + 2, h * 128:h * 128 + HD],
                t8_sb[:, 2 * jp:2 * jp + 2, h * 128 + HD:(h + 1) * 128],
                perf_mode=DR,
                start=(jp == 0),
                stop=(jp == 1),
            )

    # ---- norms: n2 = sum_ci pn (DVE pre-accumulate, then 2 ones-matmuls) --
    pn_acc = persist.tile([128, 2 * C], F32)
    nc.vector.tensor_add(pn_acc, pn_sb[:, 0, :], pn_sb[:, 1, :])
    nc.vector.tensor_add(pn_acc, pn_acc, pn_sb[:, 2, :])
    nc.vector.tensor_add(pn_acc, pn_acc, pn_sb[:, 3, :])
    pn_acc_b = persist.tile([128, 2 * C], BF16)
    nc.vector.tensor_copy(pn_acc_b, pn_acc)
    nrm_ps = [
        psum.tile([1, C], F32, tag="work_ps", name=f"nrm_ps{half}")
        for half in range(2)
    ]
    for half in range(2):
        nc.tensor.matmul(
            nrm_ps[half],
            ones_sb,
            pn_acc_b[:, half * C:(half + 1) * C],
            start=True,
            stop=True,
        )

    # ---- norms chain: r = 1/||.|| = exp(-0.5*ln(n^2)) ---------------------
    # ACT-only; the 1/64 scale on n2 cancels against the 1/64 on qk.
    lnr = persist.tile([1, 2 * C], F32)
    r_row = persist.tile([1, 2 * C], BF16)
    for half in range(2):
        nc.scalar.activation(lnr[:, half * C:(half + 1) * C], nrm_ps[half], AF.Ln)
    keep(lnr[0:1, 0:1])
    dense(2)
    for half in range(2):
        nc.scalar.activation(
            r_row[:, half * C:(half + 1) * C],
            lnr[:, half * C:(half + 1) * C],
            AF.Exp,
            scale=-0.5,
        )
    keep(r_row[0:1, 1:2])
    dense(1)

    # r -> per-partition rq via 8 tiny PE transposes, and free-dim-broadcast
    # rk via a K=1 outer-product matmul.
    ident1 = persist.tile([1, 1], BF16)
    nc.vector.memset(ident1, 1.0)
    ones64 = persist.tile([1, HD], BF16)
    nc.vector.memset(ones64, 1.0)
    tr_ps = psum.tile([128, 2 * NH], BF16, tag="work_ps")
    for h in range(NH):
        nc.tensor.transpose(
            tr_ps[:, 2 * h:2 * h + 1], r_row[0:1, h * 128:(h + 1) * 128], ident1
        )
    rq_sb = persist.tile([128, NH], F32)
    tr_view = tr_ps.rearrange("p (h two) -> p h two", two=2)[:, :, 0]
    nc.vector.tensor_mul(rq_sb, tr_view, temp_b)  # fold temperature into rq
    rk_ps = psum.tile([64, C], F32, tag="work_ps")
    rk_src = r_row.rearrange("p (h s) -> p h s", s=128)[:, :, HD:]
    nc.tensor.matmul(rk_ps, ones64, rk_src, start=True, stop=True)
    rk_sb = persist.tile([64, NH, HD], F32)
    nc.scalar.copy(rk_sb, rk_ps)

    # ---- softmax -> M -> P (head-pair packed), in two head-groups ---------
    # |logits| <= max(temperature) so exp() is safe without max-subtraction.
    # Group 1's softmax overlaps group 0's M/P matmuls.
    #
    # Engines can't shift partitions, so the pair packing happens inside the
    # M matmuls: attn is written into a zero-padded stationary layout
    # attn_pad[:, j, two, two*64:(two+1)*64] and the two matmuls of pair j
    # accumulate M_even into PSUM partitions 0:64 and M_odd into 64:128 of
    # one [128, C] tile.  P then contracts 128 rows (2 heads) per matmul.
    GH = NH // 2
    lg = persist.tile([64, NH, HD], F32)
    ex = persist.tile([64, NH, HD], F32)
    ssum = persist.tile([64, NH], F32)
    attn_pad = persist.tile([64, GP, 2, 2 * HD], BF16)
    nc.vector.memset(attn_pad, 0.0)
    m2_sb = persist.tile([128, GP, C], BF16)  # [(two,d), pair, e]
    p_ps = [
        psum.tile([128, C], F32, tag="work_ps", name=f"p_ps{t}") for t in range(KC)
    ]

    def emit_p(j):  # accumulate head-pair j into all four P row-blocks
        for t in range(KC):
            nc.tensor.matmul(
                p_ps[t],
                wv2_sb[:, j, t * 128:(t + 1) * 128],
                m2_sb[:, j, :],
                start=(j == 0),
                stop=(j == GP - 1),
            )

    for grp in range(2):
        h0 = grp * GH
        for h in range(h0, h0 + GH):
            nc.vector.scalar_tensor_tensor(
                out=lg[:, h, :],
                in0=qk_ps[:, h, :],
                scalar=rq_sb[0:64, h:h + 1],
                in1=rk_sb[:, h, :],
                op0=ALU.mult,
                op1=ALU.mult,
            )
        if grp == 0:
            keep(lg[0:1, h0 + GH - 1, 0:1])
            dense(1)
        nc.scalar.activation(
            ex[:, h0:h0 + GH, :], lg[:, h0:h0 + GH, :], AF.Exp
        )
        if grp == 0:
            keep(ex[0:1, h0 + GH - 1, 0:1])
            dense(1)
        nc.vector.tensor_reduce(
            ssum[:, h0:h0 + GH, None], ex[:, h0:h0 + GH, :],
            axis=mybir.AxisListType.X, op=ALU.add,
        )
        nc.vector.reciprocal(ssum[:, h0:h0 + GH], ssum[:, h0:h0 + GH])
        for h in range(h0, h0 + GH):
            two, j = h % 2, h // 2
            nc.vector.tensor_mul(
                attn_pad[:, j, two, two * HD:(two + 1) * HD],
                ex[:, h, :],
                ssum[:, h:h + 1].broadcast_to([64, HD]),
            )
        # M/P software pipeline: P(pair j-1) runs while m2[j] is being filled
        for j in range(grp * GH // 2, grp * GH // 2 + GH // 2):
            m2_ps = psum.tile([128, C], F32, tag="work_ps")
            for two in range(2):
                nc.tensor.matmul(
                    m2_ps,
                    attn_pad[:, j, two, :],
                    wp_sb[:, 2 * j + two, :],
                    start=(two == 0),
                    stop=(two == 1),
                )
            if j % 2 == 0:
                nc.scalar.copy(m2_sb[:, j, :], m2_ps)
            else:
                nc.vector.tensor_copy(m2_sb[:, j, :], m2_ps)
            if j > 0:
                emit_p(j - 1)
    emit_p(GP - 1)

    p_sb = persist.tile([128, KC, C], BF16)
    for t in range(KC):
        if t % 2 == 0:
            nc.scalar.copy(p_sb[:, t, :], p_ps[t])
        else:
            nc.vector.tensor_copy(p_sb[:, t, :], p_ps[t])

    # ---- y = x @ P + b ----------------------------------------------------
    # paired-tile stores: one DMA per two n-tiles, alternating queues.
    y_paired = y[:].rearrange("(t p) c -> p t c", p=128)
    ybuf = None
    for t in range(NT):
        y_ps = psum.tile([128, C], F32, tag="work_ps")
        for kc in range(KC):
            nc.tensor.matmul(
                y_ps,
                xt_sb[:, kc, t * 128:(t + 1) * 128],
                p_sb[:, kc, :],
                start=(kc == 0),
                stop=(kc == KC - 1),
            )
        if t % 2 == 0:
            ybuf = ypool.tile([128, 2, C], BF16)
        nc.vector.tensor_add(ybuf[:, t % 2, :], y_ps, bias_sb)
        if t % 2 == 1:
            eng = nc.sync if t % 4 == 1 else nc.scalar
            eng.dma_start(out=y_paired[:, t - 1:t + 1, :], in_=ybuf)


def build_nc():
    nc = bacc.Bacc("TRN2", target_bir_lowering=False, debug=False, num_devices=B)
    io = {}
    io["x_nat"] = nc.dram_tensor("x_nat", [128, NT, C], FP8, kind="ExternalInput")
    io["x_tr"] = nc.dram_tensor("x_tr", [C, N], BF16, kind="ExternalInput")
    io["wqk8"] = nc.dram_tensor("wqk8", [128, KC, 2 * C], FP8, kind="ExternalInput")
    io["wqkb"] = nc.dram_tensor("wqkb", [128, KC, 2 * C], BF16, kind="ExternalInput")
    io["wv2"] = nc.dram_tensor("wv2", [128, GP, C], BF16, kind="ExternalInput")
    io["wp2"] = nc.dram_tensor("wp2", [64, NH, C], BF16, kind="ExternalInput")
    io["bpr"] = nc.dram_tensor("bpr", [C], F32, kind="ExternalInput")
    io["temp"] = nc.dram_tensor("temp", [NH], F32, kind="ExternalInput")
    io["y"] = nc.dram_tensor("y", [N, C], BF16, kind="ExternalOutput")
    with tile.TileContext(nc) as tc:
        with ExitStack() as ctx:
            _build_kernel_body(ctx, tc, io)
    nc.compile()
    return nc


_NC_CACHE = None


def _get_nc():
    global _NC_CACHE
    if _NC_CACHE is None:
        _NC_CACHE = build_nc()
    return _NC_CACHE


def prep_host_inputs(x, W_qkv, temperature, W_proj, b_proj):
    """Host-side preprocessing shared by all cores. Returns per-core in_maps."""
    x = np.asarray(x, dtype=np.float32)
    W_qkv = np.asarray(W_qkv, dtype=np.float32)
    temperature = np.asarray(temperature, dtype=np.float32).reshape(NH)
    W_proj = np.asarray(W_proj, dtype=np.float32)
    b_proj = np.asarray(b_proj, dtype=np.float32)

    Wq = W_qkv[:, 0:C].reshape(C, NH, HD)
    Wk = W_qkv[:, C:2 * C].reshape(C, NH, HD)
    wqk_perm = np.concatenate([Wq, Wk], axis=2).reshape(C, 2 * C)  # [(ci),(h)(qk c)]
    wqk_tiled = np.ascontiguousarray(
        wqk_perm.reshape(KC, 128, 2 * C).transpose(1, 0, 2)
    )  # [p, kc, 2C]
    Wv = W_qkv[:, 2 * C:3 * C]  # [ci, (h d)]
    # [(two,d), pair, ci] so head-pairs stack on the partition dim for P
    wv2 = np.ascontiguousarray(
        Wv.T.reshape(GP, 2, HD, C).transpose(1, 2, 0, 3).reshape(128, GP, C)
    )
    wp2 = np.ascontiguousarray(
        W_proj.reshape(NH, HD, C).transpose(1, 0, 2)
    )  # [c, h, e]

    wqk8 = wqk_tiled.astype(FP8_NP)
    wqkb = wqk_tiled.astype(BF16_NP)
    wv2_bf = wv2.astype(BF16_NP)
    wp2_bf = wp2.astype(BF16_NP)

    in_maps = []
    for b in range(B):
        xb = x[b]
        in_maps.append({
            "x_nat": np.ascontiguousarray(
                (xb * 0.125).reshape(NT, 128, C).transpose(1, 0, 2)
            ).astype(FP8_NP),
            "x_tr": np.ascontiguousarray(xb.T).astype(BF16_NP),
            "wqk8": wqk8,
            "wqkb": wqkb,
            "wv2": wv2_bf,
            "wp2": wp2_bf,
            "bpr": b_proj,
            "temp": temperature,
        })
    return in_maps


def kernel(**inputs):
    x = inputs["x"]
    in_maps = prep_host_inputs(
        x, inputs["W_qkv"], inputs["temperature"], inputs["W_proj"], inputs["b_proj"]
    )
    nc = _get_nc()
    res = run_bass_kernel_spmd(nc, in_maps, list(range(B)))
    y = np.stack([np.asarray(res.results[i]["y"]) for i in range(B)], axis=0)
    return y.astype(np.float32)


if __name__ == "__main__":
    # smoke test with random data
    rng = np.random.default_rng(0)
    ins = {
        "x": rng.standard_normal((B, N, C), dtype=np.float32),
        "x_out": rng.standard_normal((B, N, C), dtype=np.float32),
        "W_qkv": (rng.standard_normal((C, 3 * C), dtype=np.float32) / np.sqrt(C)),
        "temperature": np.ones((NH, 1, 1), np.float32),
        "W_proj": (rng.standard_normal((C, C), dtype=np.float32) / np.sqrt(C)),
        "b_proj": rng.standard_normal((C,), dtype=np.float32) * 0.01,
        "H": 64,
        "W": 64,
    }
    out = kernel(**ins)
    print("out", out.shape, out.dtype, float(np.abs(out).max()))
